# revision 11
# baseline (speedup 1.0000x reference)
"""Trainium2 Bass kernel for nn_MultiHeadCausalAttention (B=4, S=2048, D=1024, H=16).

Sharding: 8 cores = 4 (batch) x 2 (tensor-parallel over heads; 8 heads/core).
Per core:
  - QKV projections for its 8 heads, computed from x^T (host-transposed).
  - Flash-style causal attention in transposed-score layout: S^T = K @ Q^T with
    k on partitions, so exp(S^T) feeds the P^T @ V matmul directly (contraction
    over k) with no on-chip transposes.
  - Softmax denominators come from col-tiled companion matmuls: each AV step
    issues 4 M=64 matmuls - A_h0 (rows 0-63) and A_h1 (rows 64-127) into an
    "A" bank, plus ones-weighted copies d_h0/d_h1 into a "d" bank at the SAME
    partition ranges.  The denominator therefore lands broadcast across the
    partitions of its head, so normalization is one full-width DVE reciprocal
    + one multiply.
  - AllGather (pairs) of the per-head attention outputs A^T so each core can
    apply the full out-projection for its 512 output columns (host-sliced Wo).

Scheduling: the attention phase is ACT(exp)-bound (~1.15us/step), leaving the
PE ~40% idle.  All other matmul work is interleaved INTO the attention steps
of the in-NEFF replay loop:
  - qk3 projections of rep r (steps ~2-25 of attention(r); QT[3] is free
    after attention(r-1) and not read again until step 120),
  - out-projection of rep r-1 (steps ~30-75; its AllGather finished during
    early attention(r)),
  - v/qk0/qk1/qk2 projections of rep r+1 (steps ~58-155, after the x^T
    re-stream for r+1 completes; VS is double-buffered by rep parity, QT[hp]
    frees after hp's last attention step).
PSUM banks: 0-3 score slots (double-buffered), 4 = AV accumulators,
5 = denominator accumulators, 6-7 = rotating chain pool shared by projection
and out-projection accumulation chains (paced against their DVE consumers).

Raw Bass (no Tile): per-engine programs with hand-placed counting semaphores.
`reps` replays the body inside one NEFF (sem values offset per rep) so the
true per-iteration time can be measured as a slope, independent of the ~78 ms
axon dispatch floor.
Host: transposes x, slices/casts weights to bf16, assembles the output halves.
"""

from contextlib import ExitStack

import numpy as np
import ml_dtypes

import concourse.bass as bass
import concourse.mybir as mybir
from concourse.bass_utils import run_bass_kernel_spmd

F32 = mybir.dt.float32
BF16 = mybir.dt.bfloat16
AF = mybir.ActivationFunctionType

B, S_FULL, D = 4, 2048, 1024
NCORES = 8
NDT = D // 128
DOWN = D // 2          # output dims owned per core (8 heads * 64)
SCALE = 1.0 / 32.0     # d_out ** -0.5
RG = [[0, 1], [2, 3], [4, 5], [6, 7]]


class Waiter:
    """Per-engine wait helper that elides waits already implied."""

    def __init__(self, eng):
        self.eng = eng
        self.seen = {}

    def __call__(self, sem, val):
        if val <= 0:
            return
        if self.seen.get(sem.name, -1) >= val:
            return
        self.seen[sem.name] = val
        self.eng.wait_ge(sem, val)


def build_program(S=S_FULL, reps=1):
    NQB = S // 512
    NST = S // 128
    NHP = 4
    NBLK = NHP * NQB

    # attention step list
    steps = []
    for hp in range(NHP):
        for qb in range(NQB):
            nkt = (qb + 1) * 4
            for kt in range(nkt):
                steps.append((hp, qb, kt, nkt, kt * 128 - qb * 512))
    NSTEPS = len(steps)
    hp_last = [max(i for i, s in enumerate(steps) if s[0] == hp) for hp in range(NHP)]

    # proj item order: v x16, then qk per head-pair (j: v 0-15, qk_hp at
    # 16+8*hp).  Global item n = rep*NPJ + j matches emission order:
    # rep r's items 0-39 are interleaved into attention(r-1), items 40-47
    # (qk3) into attention(r).
    proj_items = [("v", st) for st in range(NST)]
    for hp in range(NHP):
        for sb in range(NQB):
            proj_items.append(("q", hp, sb))
            proj_items.append(("k", hp, sb))
    NPJ = len(proj_items)

    def attn_events(r):
        """Chain events interleaved into attention(r): step -> [event].
        event = ("pj", rr, j) or ("op", rop, qt)."""
        ev = {}

        def put(i, e):
            ev.setdefault(min(i, NSTEPS - 1), []).append(e)

        if r >= 1:
            for k in range(8):                 # qk3 of rep r
                put(2 + 3 * k, ("pj", r, 40 + k))
            for qt in range(NST):              # out-projection of rep r-1
                put(30 + 3 * qt, ("op", r - 1, qt))
        if r + 1 < reps:
            for k in range(40):                # v/qk0-2 of rep r+1
                put(58 + int(k * 2.45), ("pj", r + 1, k))
        return ev

    nc = bass.Bass()
    xt = nc.declare_dram_parameter("xt", [D, S], BF16, isOutput=False)
    wq = nc.declare_dram_parameter("wq", [D, DOWN], BF16, isOutput=False)
    wk = nc.declare_dram_parameter("wk", [D, DOWN], BF16, isOutput=False)
    wv = nc.declare_dram_parameter("wv", [D, DOWN], BF16, isOutput=False)
    wo = nc.declare_dram_parameter("wo", [D, DOWN], BF16, isOutput=False)
    bob = nc.declare_dram_parameter("bob", [128, DOWN], F32, isOutput=False)
    ntri = nc.declare_dram_parameter("ntri", [128, 896], BF16, isOutput=False)
    negi = nc.declare_dram_parameter("negi", [128, 128], BF16, isOutput=False)
    out = nc.declare_dram_parameter("out", [S, DOWN], F32, isOutput=True)

    cci = [nc.dram_tensor(f"cci{i}", [128, S], BF16) for i in range(NHP)]
    cco = [nc.dram_tensor(f"cco{i}", [256, S], BF16) for i in range(NHP)]

    with ExitStack() as ctx:
        e = ctx.enter_context
        ctx.enter_context(
            nc.allow_low_precision(reason="intentional bf16 flash attention")
        )

        sems = {}
        for n in (
            "dXQ", "dWK", "dWV", "dWO", "dMISC", "sPJ", "sPJC", "sPS", "sEX",
            "sAV", "sA", "dCC", "sCG", "dAT", "sOP", "sOB",
            "dO0", "dO1", "dO2", "dO3", "sON",
        ):
            sems[n] = e(nc.semaphore(n))
        dXQ, dWK, dWV, dWO, dMISC = (sems[k] for k in ("dXQ", "dWK", "dWV", "dWO", "dMISC"))
        sPJ, sPJC, sPS, sEX = (sems[k] for k in ("sPJ", "sPJC", "sPS", "sEX"))
        sAV, sA = (sems[k] for k in ("sAV", "sA"))
        dCC, sCG, dAT, sOP, sOB = (sems[k] for k in ("dCC", "sCG", "dAT", "sOP", "sOB"))
        sON = sems["sON"]
        dO = [sems[f"dO{i}"] for i in range(4)]

        # PSUM banks: 0-3 score slots, 4 AV, 5 denominators, 6-7 chain pool
        P = e(nc.psum_tensor("P", [128, 4096], F32))
        AB, DB = 2048, 2560

        QT = [e(nc.sbuf_tensor(f"QT{i}", [128, S], BF16)) for i in range(NHP)]
        KT = [e(nc.sbuf_tensor(f"KT{i}", [128, S], BF16)) for i in range(NHP)]
        VSD = [
            [e(nc.sbuf_tensor(f"VS{p}_{i}", [128, 512], BF16)) for i in range(NST)]
            for p in range(2)
        ]
        WOt = [e(nc.sbuf_tensor(f"WOt{i}", [128, DOWN], BF16)) for i in range(NDT)]
        ntri_sb = e(nc.sbuf_tensor("ntri_sb", [128, 896], BF16))
        negi_sb = e(nc.sbuf_tensor("negi_sb", [128, 128], BF16))
        ones_sb = e(nc.sbuf_tensor("ones_sb", [128, 64], BF16))
        bob_sb = e(nc.sbuf_tensor("bob_sb", [128, DOWN], F32))
        PT = [e(nc.sbuf_tensor(f"PT{i}", [128, 1024], BF16)) for i in range(6)]
        RSB = e(nc.sbuf_tensor("RSB", [128, 512], F32))
        OSB = [e(nc.sbuf_tensor(f"OSB{i}", [128, DOWN], F32)) for i in range(4)]
        XT = [e(nc.sbuf_tensor(f"XT{i}", [128, S], BF16)) for i in range(NDT)]
        WQt = [e(nc.sbuf_tensor(f"WQt{i}", [128, DOWN], BF16)) for i in range(NDT)]
        WKt = [e(nc.sbuf_tensor(f"WKt{i}", [128, DOWN], BF16)) for i in range(NDT)]
        WVt = [e(nc.sbuf_tensor(f"WVt{i}", [128, DOWN], BF16)) for i in range(NDT)]
        ATB = [e(nc.sbuf_tensor(f"ATB{i}", [128, S], BF16)) for i in range(NDT)]
        # A^T head pairs: head 2hp+r lives on partitions r*64:(r+1)*64
        ASBP = [e(nc.sbuf_tensor(f"ASBP{i}", [128, S], BF16)) for i in range(NHP)]

        # shared chain-pool bookkeeping (banks 6-7): both PE and DVE emitters
        # walk the chains in the same global order, so bank = index % 2 and
        # the PE paces against the consumer of the chain two back.
        chain_hist = []  # ("pj", n) or ("op", gq)

        with nc.Block() as blk:

            @blk.sync
            def _(sync):
                w = Waiter(sync)
                for r in range(reps):
                    if r == 0:
                        for i in range(NDT):
                            sl = slice(i * 128, (i + 1) * 128)
                            sync.dma_start(XT[i][:], xt[sl, :]).then_inc(dXQ, 16)
                            sync.dma_start(WQt[i][:], wq[sl, :]).then_inc(dXQ, 16)
                        for i in range(NDT):
                            sl = slice(i * 128, (i + 1) * 128)
                            sync.dma_start(WKt[i][:], wk[sl, :]).then_inc(dWK, 16)
                        for i in range(NDT):
                            sl = slice(i * 128, (i + 1) * 128)
                            sync.dma_start(WVt[i][:], wv[sl, :]).then_inc(dWV, 16)
                        sync.dma_start(ntri_sb[:], ntri[:]).then_inc(dMISC, 16)
                        sync.dma_start(negi_sb[:], negi[:]).then_inc(dMISC, 16)
                        sync.dma_start(bob_sb[:], bob[:]).then_inc(dMISC, 16)
                        for i in range(NDT):
                            sl = slice(i * 128, (i + 1) * 128)
                            sync.dma_start(WOt[i][:], wo[sl, :]).then_inc(dWO, 16)
                    if r + 1 < reps:
                        # re-stream x^T for rep r+1 (tiles 0-3 here, 4-7 on
                        # the gpsimd queue) as soon as rep r's projection
                        # matmuls are done reading it.
                        w(sPJ, (r + 1) * NPJ)
                        for i in range(4):
                            sl = slice(i * 128, (i + 1) * 128)
                            sync.dma_start(XT[i][:], xt[sl, :]).then_inc(dXQ, 16)
                    if r >= 1:
                        # stores of rep r-1's out-projection (interleaved in
                        # attention(r) steps ~30-75)
                        for qt in range(NST):
                            gq = (r - 1) * NST + qt
                            w(sOB, gq + 1)
                            sync.dma_start(
                                out[qt * 128 : (qt + 1) * 128, :], OSB[qt % 4][:]
                            ).then_inc(dO[qt % 4], 16)
                    for hp in range(NHP):
                        w(sA, r * NBLK + hp * NQB + NQB)
                        if r > 0:
                            w(sCG, NHP * (r - 1) + hp + 1)  # cci free
                        sync.dma_start(cci[hp][:], ASBP[hp][:]).then_inc(dCC, 16)
                # final rep's stores
                for qt in range(NST):
                    gq = (reps - 1) * NST + qt
                    w(sOB, gq + 1)
                    sync.dma_start(
                        out[qt * 128 : (qt + 1) * 128, :], OSB[qt % 4][:]
                    ).then_inc(dO[qt % 4], 16)
                for i in range(4):
                    w(dO[i], 16 * reps * 4)

            @blk.gpsimd
            def _(gpsimd):
                w = Waiter(gpsimd)
                for r in range(reps):
                    if r + 1 < reps:
                        w(sPJ, (r + 1) * NPJ)
                        for i in range(4, NDT):
                            sl = slice(i * 128, (i + 1) * 128)
                            gpsimd.dma_start(XT[i][:], xt[sl, :]).then_inc(
                                dXQ, 16
                            )
                    for hp in range(NHP):
                        w(dCC, 16 * (NHP * r + hp + 1))
                        if r > 0:
                            w(dAT, 32 * (NHP * (r - 1) + hp + 1))  # cco free
                        gpsimd.collective_compute(
                            "AllGather",
                            mybir.AluOpType.bypass,
                            replica_groups=RG,
                            ins=[cci[hp][:]],
                            outs=[cco[hp][:]],
                        ).then_inc(sCG, 1)
                        w(sCG, NHP * r + hp + 1)
                        if r > 0:
                            w(sOP, r * NST)  # ATB free (prev outproj done)
                        gpsimd.dma_start(ATB[hp][:], cco[hp][0:128, :]).then_inc(
                            dAT, 16
                        )
                        gpsimd.dma_start(
                            ATB[hp + 4][:], cco[hp][128:256, :]
                        ).then_inc(dAT, 16)

            @blk.tensor
            def _(tensor):
                w = Waiter(tensor)

                def chain_wait():
                    if len(chain_hist) >= 2:
                        kind, idx = chain_hist[-2]
                        if kind == "pj":
                            w(sPJC, idx + 1)
                        else:
                            w(sOB, idx + 1)

                def pe_proj_chain(rr, j):
                    n = rr * NPJ + j
                    chain_wait()
                    bank = 6 + (len(chain_hist) % 2)
                    chain_hist.append(("pj", n))
                    pslc = slice(bank * 512, bank * 512 + 512)
                    item = proj_items[j]
                    if item[0] == "v":
                        _, st = item
                        stsl = slice(st * 128, (st + 1) * 128)
                        for dt in range(NDT):
                            w(dXQ, 256 + 128 * rr)
                            w(dWV, 128)
                            mm = nc.tensor.matmul(
                                P[:, pslc],
                                lhsT=XT[dt][:, stsl],
                                rhs=WVt[dt][:],
                                start=(dt == 0),
                                stop=(dt == NDT - 1),
                                skip_group_check=True,
                            )
                        mm.then_inc(sPJ, 1)
                    else:
                        kind, hp, sb = item
                        wt = WQt if kind == "q" else WKt
                        hsl = slice(hp * 128, (hp + 1) * 128)
                        ssl = slice(sb * 512, (sb + 1) * 512)
                        for dt in range(NDT):
                            w(dXQ, 256 + 128 * rr)
                            if kind == "k":
                                w(dWK, 128)
                            mm = nc.tensor.matmul(
                                P[:, pslc],
                                lhsT=wt[dt][:, hsl],
                                rhs=XT[dt][:, ssl],
                                start=(dt == 0),
                                stop=(dt == NDT - 1),
                                skip_group_check=True,
                            )
                        mm.then_inc(sPJ, 1)

                OPORDER = [0, 4, 1, 5, 2, 6, 3, 7]

                def pe_outproj_chain(rop, qt):
                    gq = rop * NST + qt
                    w(dWO, 128)
                    chain_wait()
                    bank = 6 + (len(chain_hist) % 2)
                    chain_hist.append(("op", gq))
                    qsl = slice(qt * 128, (qt + 1) * 128)
                    for pos, dtk in enumerate(OPORDER):
                        w(dAT, 32 * (NHP * rop + (dtk % 4) + 1))
                        mm = nc.tensor.matmul(
                            P[:, bank * 512 : bank * 512 + 512],
                            lhsT=ATB[dtk][:, qsl],
                            rhs=WOt[dtk][:],
                            start=(pos == 0),
                            stop=(pos == 7),
                            skip_group_check=True,
                        )
                    mm.then_inc(sOP, 1)

                def pe_event(ev):
                    if ev[0] == "pj":
                        pe_proj_chain(ev[1], ev[2])
                    else:
                        pe_outproj_chain(ev[1], ev[2])

                def emit_attention(r):
                    evs = attn_events(r)
                    w(sPJC, r * NPJ + 24)  # v + qk0 of rep r copied
                    w(dMISC, 48)
                    if r == 0:
                        w(sON, 1)

                    def emit_scores(i):
                        hp, qb, kt, nkt, delta = steps[i]
                        gi = r * NSTEPS + i
                        s = i % 2
                        if i == hp * (NSTEPS // NHP):
                            w(sPJC, r * NPJ + 24 + 8 * hp)
                        qsl = slice(qb * 512, (qb + 1) * 512)
                        ksl = slice(kt * 128, (kt + 1) * 128)
                        w(sEX, gi - 1)
                        diag = delta >= 0
                        for rr in range(2):
                            psl = slice(rr * 64, (rr + 1) * 64)
                            mm = nc.tensor.matmul(
                                P[:, s * 1024 + rr * 512 : s * 1024 + rr * 512 + 512],
                                lhsT=KT[hp][psl, ksl],
                                rhs=QT[hp][psl, qsl],
                                start=True,
                                stop=not diag,
                                tile_position=(rr * 64, 0),
                                skip_group_check=True,
                            )
                        if diag:
                            # causal mask: accumulate -BIG onto j < delta + p
                            wsl = slice(384, 896 - delta)
                            for rr in range(2):
                                base = s * 1024 + rr * 512
                                mm = nc.tensor.matmul(
                                    P[:, base + delta : base + 512],
                                    lhsT=negi_sb[:],
                                    rhs=ntri_sb[:, wsl],
                                    start=False,
                                    stop=True,
                                    skip_group_check=True,
                                )
                        mm.then_inc(sPS, 1)

                    emit_scores(0)
                    for i, (hp, qb, kt, nkt, delta) in enumerate(steps):
                        gi = r * NSTEPS + i
                        gblk = r * NBLK + hp * NQB + qb
                        w0 = max(delta, 0)
                        if i + 1 < NSTEPS:
                            emit_scores(i + 1)
                        # interleaved chains go BEFORE the exp wait: they run
                        # while ACT is still computing exp(i), and never delay
                        # the next scores emission (which gates the exp
                        # stream); the AV lag they cause is absorbed by the
                        # 6-deep PT rotation.
                        for ev in evs.get(i, ()):
                            pe_event(ev)
                        w(sEX, gi + 1)
                        # banks 4-5 free once block gblk-1's mul is done
                        w(sA, gblk)
                        first = kt == 0
                        last = kt == nkt - 1
                        h0 = 2 * hp
                        pt = PT[i % 6]
                        vs = VSD[r % 2][kt]
                        mm = nc.tensor.matmul(
                            P[0:64, AB + w0 : AB + 512],
                            lhsT=vs[:, h0 * 64 : h0 * 64 + 64],
                            rhs=pt[:, w0:512],
                            start=first,
                            stop=last,
                            skip_group_check=True,
                        )
                        mm = nc.tensor.matmul(
                            P[64:128, AB + w0 : AB + 512],
                            lhsT=vs[:, h0 * 64 + 64 : h0 * 64 + 128],
                            rhs=pt[:, 512 + w0 : 1024],
                            start=first,
                            stop=last,
                            skip_group_check=True,
                        )
                        mm = nc.tensor.matmul(
                            P[0:64, DB + w0 : DB + 512],
                            lhsT=ones_sb[:],
                            rhs=pt[:, w0:512],
                            start=first,
                            stop=last,
                            skip_group_check=True,
                        )
                        mm = nc.tensor.matmul(
                            P[64:128, DB + w0 : DB + 512],
                            lhsT=ones_sb[:],
                            rhs=pt[:, 512 + w0 : 1024],
                            start=first,
                            stop=last,
                            skip_group_check=True,
                        )
                        mm.then_inc(sAV, 1)

                # prefix: all of proj(0) ahead of attention(0)
                for j in range(NPJ):
                    pe_proj_chain(0, j)
                for r in range(reps):
                    emit_attention(r)
                # final rep's out-projection
                for qt in range(NST):
                    pe_outproj_chain(reps - 1, qt)

            @blk.scalar
            def _(scalar):
                w = Waiter(scalar)
                for r in range(reps):
                    for i, (hp, qb, kt, nkt, delta) in enumerate(steps):
                        gi = r * NSTEPS + i
                        w0 = max(delta, 0)
                        s = i % 2
                        w(sPS, gi + 1)
                        w(sAV, gi - 5)
                        src = P[:, s * 1024 : (s + 1) * 1024]
                        dst = PT[i % 6][:, :]
                        if w0 == 0:
                            act = nc.scalar.activation(dst, src, AF.Exp, scale=SCALE)
                        else:
                            sv = src.rearrange("p (t c) -> p t c", t=2)[:, :, w0:512]
                            dv = dst.rearrange("p (t c) -> p t c", t=2)[:, :, w0:512]
                            act = nc.scalar.activation(dv, sv, AF.Exp, scale=SCALE)
                        act.then_inc(sEX, 1)

            @blk.vector
            def _(vector):
                w = Waiter(vector)
                nc.vector.memset(ones_sb[:], 1.0).then_inc(sON, 1)
                dve_chain = [0]  # mirrors chain_hist bank rotation

                def dve_proj_copy(rr, j):
                    n = rr * NPJ + j
                    bank = 6 + (dve_chain[0] % 2)
                    dve_chain[0] += 1
                    pslc = slice(bank * 512, bank * 512 + 512)
                    w(sPJ, n + 1)
                    item = proj_items[j]
                    if item[0] == "v":
                        _, st = item
                        nc.vector.tensor_copy(
                            VSD[rr % 2][st][:, :], P[:, pslc]
                        ).then_inc(sPJC, 1)
                    else:
                        kind, hp, sb = item
                        if rr > 0:
                            w(sAV, (rr - 1) * NSTEPS + hp_last[hp] + 1)
                        dst = (QT if kind == "q" else KT)[hp]
                        ssl = slice(sb * 512, (sb + 1) * 512)
                        nc.vector.tensor_copy(dst[:, ssl], P[:, pslc]).then_inc(
                            sPJC, 1
                        )

                def dve_out_add(rop, qt):
                    gq = rop * NST + qt
                    bank = 6 + (dve_chain[0] % 2)
                    dve_chain[0] += 1
                    w(sOP, gq + 1)
                    if gq >= 4:
                        w(dO[qt % 4], 16 * (rop * 4 + qt // 4))
                    nc.vector.tensor_add(
                        OSB[qt % 4][:],
                        P[:, bank * 512 : bank * 512 + 512],
                        bob_sb[:],
                    ).then_inc(sOB, 1)

                def dve_event(ev):
                    if ev[0] == "pj":
                        dve_proj_copy(ev[1], ev[2])
                    else:
                        dve_out_add(ev[1], ev[2])

                for j in range(NPJ):
                    dve_proj_copy(0, j)
                for r in range(reps):
                    evs = attn_events(r)
                    for i, (hp, qb, kt, nkt, delta) in enumerate(steps):
                        gi = r * NSTEPS + i
                        for ev in evs.get(i, ()):
                            dve_event(ev)
                        if kt == nkt - 1:
                            qsl = slice(qb * 512, (qb + 1) * 512)
                            w(sAV, gi + 1)
                            if r > 0:
                                w(dCC, 16 * (NHP * (r - 1) + hp + 1))
                            nc.vector.reciprocal(RSB[:], P[:, DB : DB + 512])
                            nc.vector.tensor_mul(
                                ASBP[hp][:, qsl],
                                P[:, AB : AB + 512],
                                RSB[:],
                            ).then_inc(sA, 1)
                for qt in range(NST):
                    dve_out_add(reps - 1, qt)

    return nc


_cached = {}


def _get_program(S=S_FULL, reps=1):
    key = (S, reps)
    if key not in _cached:
        _cached[key] = build_program(S, reps)
    return _cached[key]


def make_in_maps(x, Wq, Wk, Wv, Wo, bo):
    bf = ml_dtypes.bfloat16
    ntri01 = (np.arange(896)[None, :] < (np.arange(128)[:, None] + 384)).astype(bf)
    negi01 = (np.eye(128) * -60000.0).astype(bf)
    x = np.asarray(x)
    # each batch's transposed activations feed both TP halves: build once
    xtb = [np.ascontiguousarray(x[b].T).astype(bf) for b in range(B)]
    in_maps = []
    for c in range(NCORES):
        b, p = divmod(c, 2)
        dsl = slice(p * DOWN, (p + 1) * DOWN)
        in_maps.append(
            {
                "xt": xtb[b],
                "wq": np.ascontiguousarray(np.asarray(Wq)[:, dsl]).astype(bf),
                "wk": np.ascontiguousarray(np.asarray(Wk)[:, dsl]).astype(bf),
                "wv": np.ascontiguousarray(np.asarray(Wv)[:, dsl]).astype(bf),
                "wo": np.ascontiguousarray(np.asarray(Wo)[:, dsl]).astype(bf),
                "bob": np.tile(np.asarray(bo, np.float32)[dsl], (128, 1)),
                "ntri": ntri01,
                "negi": negi01,
            }
        )
    return in_maps


def assemble(results, S):
    out = np.empty((B, S, D), np.float32)
    for c in range(NCORES):
        b, p = divmod(c, 2)
        out[b, :, p * DOWN : (p + 1) * DOWN] = results[c]["out"]
    return out


def kernel(**inputs):
    x = np.asarray(inputs["x"], np.float32)
    S = x.shape[1]
    nc = _get_program(S)
    in_maps = make_in_maps(
        x,
        inputs["Wq"],
        inputs["Wk"],
        inputs["Wv"],
        inputs["Wo"],
        inputs["bo"],
    )
    res = run_bass_kernel_spmd(nc, in_maps, core_ids=list(range(NCORES)))
    return assemble(res.results, S)


# revision 18
# speedup vs baseline: 1.0636x; 1.0636x over previous
"""Trainium2 Bass kernel for nn_MultiHeadCausalAttention (B=4, S=2048, D=1024, H=16).

Sharding: 8 cores = 4 (batch) x 2 (tensor-parallel over heads; 8 heads/core).
Per core:
  - QKV projections for its 8 heads, computed from x^T (host-transposed).
  - Flash-style causal attention in transposed-score layout: S^T = K @ Q^T with
    k on partitions, so exp(S^T) feeds the P^T @ V matmul directly (contraction
    over k) with no on-chip transposes.
  - Softmax denominators come from col-tiled companion matmuls: each AV step
    issues 4 M=64 matmuls — A_h0 (rows 0-63) and A_h1 (rows 64-127) into an
    "A" bank, plus ones-weighted copies d_h0/d_h1 into a "d" bank at the SAME
    partition ranges.  The denominator therefore lands broadcast across the
    partitions of its head, so normalization is one full-width DVE reciprocal
    + one multiply (the single-partition reciprocal + rank-1 broadcast matmul
    of the previous design was a 6.8us PE stall per block that also kept
    re-throttling the PE clock gate).  A/d bank pairs rotate per block so the
    DVE normalization runs concurrently with the next block's matmuls.
  - AllGather (pairs) of the per-head attention outputs A^T so each core can
    apply the full out-projection for its 512 output columns (host-sliced Wo).
Raw Bass (no Tile): per-engine programs with hand-placed counting semaphores.
`reps` replays the body inside one NEFF (sem values offset per rep) so the
true per-iteration time can be measured as a slope, independent of the ~78 ms
axon dispatch floor.
Host: transposes x, slices/casts weights to bf16, assembles the output halves.
"""

from contextlib import ExitStack

import numpy as np
import ml_dtypes

import concourse.bass as bass
import concourse.mybir as mybir
from concourse.bass_utils import run_bass_kernel_spmd

F32 = mybir.dt.float32
BF16 = mybir.dt.bfloat16
AF = mybir.ActivationFunctionType

B, S_FULL, D = 4, 2048, 1024
NCORES = 8
NDT = D // 128
DOWN = D // 2          # output dims owned per core (8 heads * 64)
SCALE = 1.0 / 32.0     # d_out ** -0.5
RG = [[0, 1], [2, 3], [4, 5], [6, 7]]

# How the first matmul of each per-bank accumulation chain marks start= when
# two col-tiled chains share a bank (see microtest_psum.py):
#   "both"  - every chain's first matmul uses start=True (region-scoped clear)
#   "first" - only the first chain into the bank uses start=True (whole-bank
#             clear; the second chain overwrites into cleared bits)
CHAIN_START = "both"


class Waiter:
    """Per-engine wait helper that elides waits already implied."""

    def __init__(self, eng):
        self.eng = eng
        self.seen = {}

    def __call__(self, sem, val):
        if val <= 0:
            return
        if self.seen.get(sem.name, -1) >= val:
            return
        self.seen[sem.name] = val
        self.eng.wait_ge(sem, val)


def build_program(S=S_FULL, reps=1):
    NQB = S // 512
    NST = S // 128
    NHP = 4
    NBLK = NHP * NQB

    # attention step list
    steps = []
    for hp in range(NHP):
        for qb in range(NQB):
            nkt = (qb + 1) * 4
            for kt in range(nkt):
                steps.append((hp, qb, kt, nkt, kt * 128 - qb * 512))
    NSTEPS = len(steps)
    # last attention step index touching head-pair hp / V tile st
    hp_last = [max(i for i, s in enumerate(steps) if s[0] == hp) for hp in range(NHP)]
    vs_last = [
        max(i for i, s in enumerate(steps) if s[2] == st) for st in range(NST)
    ]

    # proj emission order shared by PE and DVE
    proj = []
    for hp in range(NHP):
        for sb in range(S // 512):
            proj.append(("q", hp, sb))
            proj.append(("k", hp, sb))
    for st in range(NST):
        proj.append(("v", st))
    NPJ = len(proj)

    nc = bass.Bass()
    xt = nc.declare_dram_parameter("xt", [D, S], BF16, isOutput=False)
    wq = nc.declare_dram_parameter("wq", [D, DOWN], BF16, isOutput=False)
    wk = nc.declare_dram_parameter("wk", [D, DOWN], BF16, isOutput=False)
    wv = nc.declare_dram_parameter("wv", [D, DOWN], BF16, isOutput=False)
    wo = nc.declare_dram_parameter("wo", [D, DOWN], BF16, isOutput=False)
    bob = nc.declare_dram_parameter("bob", [128, DOWN], F32, isOutput=False)
    ntri = nc.declare_dram_parameter("ntri", [128, 896], BF16, isOutput=False)
    negi = nc.declare_dram_parameter("negi", [128, 128], BF16, isOutput=False)
    out = nc.declare_dram_parameter("out", [S, DOWN], F32, isOutput=True)

    cci = [nc.dram_tensor(f"cci{i}", [128, S], BF16) for i in range(NHP)]
    cco = [nc.dram_tensor(f"cco{i}", [256, S], BF16) for i in range(NHP)]

    with ExitStack() as ctx:
        e = ctx.enter_context
        ctx.enter_context(
            nc.allow_low_precision(reason="intentional bf16 flash attention")
        )

        sems = {}
        for n in (
            "dXQ", "dWK", "dWV", "dWO", "dMISC", "sPJ", "sPJC", "sPS", "sEX",
            "sAV", "sA", "dCC", "sCG", "dAT", "sOP", "sOB", "dO0", "dO1", "dO2", "dO3",
            "sON",
        ):
            sems[n] = e(nc.semaphore(n))
        dXQ, dWK, dWV, dWO, dMISC = (sems[k] for k in ("dXQ", "dWK", "dWV", "dWO", "dMISC"))
        sPJ, sPJC, sPS, sEX = (sems[k] for k in ("sPJ", "sPJC", "sPS", "sEX"))
        sAV, sA = (sems[k] for k in ("sAV", "sA"))
        dCC, sCG, dAT, sOP, sOB = (sems[k] for k in ("dCC", "sCG", "dAT", "sOP", "sOB"))
        sON = sems["sON"]
        dO = [sems[f"dO{i}"] for i in range(4)]

        # one PSUM tensor, manual bank layout:
        # banks 0-3: proj psums (q/k) & attention score slots & outproj (0-1)
        # banks 4-7: proj psums (v); during attention: A/d bank pairs,
        #            rotating per block (even blocks 4-5, odd blocks 6-7)
        P = e(nc.psum_tensor("P", [128, 4096], F32))

        QT = [e(nc.sbuf_tensor(f"QT{i}", [128, S], BF16)) for i in range(NHP)]
        KT = [e(nc.sbuf_tensor(f"KT{i}", [128, S], BF16)) for i in range(NHP)]
        VS = [e(nc.sbuf_tensor(f"VS{i}", [128, 512], BF16)) for i in range(NST)]
        WOt = [e(nc.sbuf_tensor(f"WOt{i}", [128, DOWN], BF16)) for i in range(NDT)]
        ntri_sb = e(nc.sbuf_tensor("ntri_sb", [128, 896], BF16))
        negi_sb = e(nc.sbuf_tensor("negi_sb", [128, 128], BF16))
        ones_sb = e(nc.sbuf_tensor("ones_sb", [128, 64], BF16))
        bob_sb = e(nc.sbuf_tensor("bob_sb", [128, DOWN], F32))
        PT = [e(nc.sbuf_tensor(f"PT{i}", [128, 1024], BF16)) for i in range(6)]
        RSB = e(nc.sbuf_tensor("RSB", [128, 512], F32))
        OSB = [e(nc.sbuf_tensor(f"OSB{i}", [128, DOWN], F32)) for i in range(4)]
        XT = [e(nc.sbuf_tensor(f"XT{i}", [128, S], BF16)) for i in range(NDT)]
        WQt = [e(nc.sbuf_tensor(f"WQt{i}", [128, DOWN], BF16)) for i in range(NDT)]
        WKt = [e(nc.sbuf_tensor(f"WKt{i}", [128, DOWN], BF16)) for i in range(NDT)]
        WVt = [e(nc.sbuf_tensor(f"WVt{i}", [128, DOWN], BF16)) for i in range(NDT)]
        ATB = [e(nc.sbuf_tensor(f"ATB{i}", [128, S], BF16)) for i in range(NDT)]
        # A^T head pairs: head 2hp+r lives on partitions r*64:(r+1)*64
        ASBP = [e(nc.sbuf_tensor(f"ASBP{i}", [128, S], BF16)) for i in range(NHP)]

        with nc.Block() as blk:

            @blk.sync
            def _(sync):
                w = Waiter(sync)
                for r in range(reps):
                    if r == 0:
                        for i in range(NDT):
                            sl = slice(i * 128, (i + 1) * 128)
                            sync.dma_start(XT[i][:], xt[sl, :]).then_inc(dXQ, 16)
                            sync.dma_start(WQt[i][:], wq[sl, :]).then_inc(dXQ, 16)
                        for i in range(NDT):
                            sl = slice(i * 128, (i + 1) * 128)
                            sync.dma_start(WKt[i][:], wk[sl, :]).then_inc(dWK, 16)
                        for i in range(NDT):
                            sl = slice(i * 128, (i + 1) * 128)
                            sync.dma_start(WVt[i][:], wv[sl, :]).then_inc(dWV, 16)
                        sync.dma_start(ntri_sb[:], ntri[:]).then_inc(dMISC, 16)
                        sync.dma_start(negi_sb[:], negi[:]).then_inc(dMISC, 16)
                        sync.dma_start(bob_sb[:], bob[:]).then_inc(dMISC, 16)
                        for i in range(NDT):
                            sl = slice(i * 128, (i + 1) * 128)
                            sync.dma_start(WOt[i][:], wo[sl, :]).then_inc(dWO, 16)
                    if r + 1 < reps:
                        # re-stream x for the next rep (steady-state
                        # measurement).  Issued BEFORE the cci stores: proj(r)
                        # is already done when attention(r) starts, so this
                        # overlaps the whole attention phase; behind cci[hp3]
                        # it would only start at attention end and stall
                        # proj(r+1).
                        w(sPJ, (r + 1) * NPJ)  # this rep's proj done reading XT
                        for i in range(NDT):
                            sl = slice(i * 128, (i + 1) * 128)
                            sync.dma_start(XT[i][:], xt[sl, :]).then_inc(dXQ, 16)
                    for hp in range(NHP):
                        w(sA, r * NBLK + hp * NQB + NQB)
                        if r > 0:
                            w(sCG, NHP * (r - 1) + hp + 1)  # cci free
                        sync.dma_start(cci[hp][:], ASBP[hp][:]).then_inc(dCC, 16)
                    for qt in range(NST):
                        gq = r * NST + qt
                        w(sOB, gq + 1)
                        sync.dma_start(
                            out[qt * 128 : (qt + 1) * 128, :], OSB[qt % 4][:]
                        ).then_inc(dO[qt % 4], 16)
                for i in range(4):
                    w(dO[i], 16 * reps * (NST // 4))

            @blk.gpsimd
            def _(gpsimd):
                w = Waiter(gpsimd)
                for r in range(reps):
                    for hp in range(NHP):
                        w(dCC, 16 * (NHP * r + hp + 1))
                        if r > 0:
                            w(dAT, 32 * (NHP * (r - 1) + hp + 1))  # cco free
                        gpsimd.collective_compute(
                            "AllGather",
                            mybir.AluOpType.bypass,
                            replica_groups=RG,
                            ins=[cci[hp][:]],
                            outs=[cco[hp][:]],
                        ).then_inc(sCG, 1)
                        w(sCG, NHP * r + hp + 1)
                        if r > 0:
                            w(sOP, r * NST)  # ATB free (prev outproj done)
                        gpsimd.dma_start(ATB[hp][:], cco[hp][0:128, :]).then_inc(
                            dAT, 16
                        )
                        gpsimd.dma_start(
                            ATB[hp + 4][:], cco[hp][128:256, :]
                        ).then_inc(dAT, 16)

            @blk.tensor
            def _(tensor):
                w = Waiter(tensor)

                def emit_proj(rr):
                    # projections for rep rr: q/k rotate banks 0-3,
                    # v rotates banks 4-5 (6-7 belong to outproj(rr-1))
                    w(sEX, rr * NSTEPS)  # prev rep's exps done w/ banks 0-3
                    if rr > 0:
                        w(sA, rr * NBLK - 1)  # block 14's mul: banks 4-5 free

                    def dxq_val(dt):
                        # bulk wait: DMA queue completions are unordered, so
                        # only the all-issued count is a sound threshold
                        return 256 + 128 * rr

                    for j, item in enumerate(proj):
                        gj = rr * NPJ + j
                        if item[0] != "v":
                            bank = j % 4
                            w(sPJC, gj - 3)
                        else:
                            bank = 4 + (j % 2)
                            w(sPJC, gj - 1)
                        pslc = slice(bank * 512, bank * 512 + 512)
                        if item[0] in ("q", "k"):
                            kind, hp, sb = item
                            wt = WQt if kind == "q" else WKt
                            hsl = slice(hp * 128, (hp + 1) * 128)
                            ssl = slice(sb * 512, (sb + 1) * 512)
                            for dt in range(NDT):
                                w(dXQ, dxq_val(dt))
                                if kind == "k":
                                    w(dWK, 128)
                                mm = nc.tensor.matmul(
                                    P[:, pslc],
                                    lhsT=wt[dt][:, hsl],
                                    rhs=XT[dt][:, ssl],
                                    start=(dt == 0),
                                    stop=(dt == NDT - 1),
                                    skip_group_check=True,
                                )
                            mm.then_inc(sPJ, 1)
                        else:
                            _, st = item
                            stsl = slice(st * 128, (st + 1) * 128)
                            for dt in range(NDT):
                                w(dXQ, dxq_val(dt))
                                w(dWV, 128)
                                mm = nc.tensor.matmul(
                                    P[:, pslc],
                                    lhsT=XT[dt][:, stsl],
                                    rhs=WVt[dt][:],
                                    start=(dt == 0),
                                    stop=(dt == NDT - 1),
                                    skip_group_check=True,
                                )
                            mm.then_inc(sPJ, 1)

                def emit_attention(r):
                    # PE software-pipelined: scores run one step ahead of
                    # the AV matmuls so ACT exp overlaps PE
                    w(sPJC, (r + 1) * NPJ)
                    w(dMISC, 48)
                    if r == 0:
                        w(sON, 1)

                    def emit_scores(i):
                        hp, qb, kt, nkt, delta = steps[i]
                        gi = r * NSTEPS + i
                        s = i % 2
                        qsl = slice(qb * 512, (qb + 1) * 512)
                        ksl = slice(kt * 128, (kt + 1) * 128)
                        w(sEX, gi - 1)
                        diag = delta >= 0
                        for rr in range(2):
                            psl = slice(rr * 64, (rr + 1) * 64)
                            mm = nc.tensor.matmul(
                                P[:, s * 1024 + rr * 512 : s * 1024 + rr * 512 + 512],
                                lhsT=KT[hp][psl, ksl],
                                rhs=QT[hp][psl, qsl],
                                start=True,
                                stop=not diag,
                                tile_position=(rr * 64, 0),
                                skip_group_check=True,
                            )
                        if diag:
                            # causal mask: accumulate -BIG onto j < delta + p
                            wsl = slice(384, 896 - delta)
                            for rr in range(2):
                                base = s * 1024 + rr * 512
                                mm = nc.tensor.matmul(
                                    P[:, base + delta : base + 512],
                                    lhsT=negi_sb[:],
                                    rhs=ntri_sb[:, wsl],
                                    start=False,
                                    stop=True,
                                    skip_group_check=True,
                                )
                        mm.then_inc(sPS, 1)

                    emit_scores(0)
                    for i, (hp, qb, kt, nkt, delta) in enumerate(steps):
                        gi = r * NSTEPS + i
                        gblk = r * NBLK + hp * NQB + qb
                        blk_idx = hp * NQB + qb
                        w0 = max(delta, 0)
                        if i + 1 < NSTEPS:
                            emit_scores(i + 1)
                        w(sEX, gi + 1)
                        # A/d bank pair for this block (rotates per block)
                        ab = 2048 + (blk_idx % 2) * 1024
                        db = ab + 512
                        # the pair is free once block gblk-2's mul is done
                        w(sA, gblk - 1)
                        if blk_idx == 1 and kt == 0:
                            # banks 6-7: prev rep's out-adds must be done
                            w(sOB, r * NST)
                        first = kt == 0
                        last = kt == nkt - 1
                        ysta = first if CHAIN_START == "both" else False
                        h0 = 2 * hp
                        pt = PT[i % 6]
                        mm = nc.tensor.matmul(
                            P[0:64, ab + w0 : ab + 512],
                            lhsT=VS[kt][:, h0 * 64 : h0 * 64 + 64],
                            rhs=pt[:, w0:512],
                            start=first,
                            stop=last,
                            skip_group_check=True,
                        )
                        mm = nc.tensor.matmul(
                            P[64:128, ab + w0 : ab + 512],
                            lhsT=VS[kt][:, h0 * 64 + 64 : h0 * 64 + 128],
                            rhs=pt[:, 512 + w0 : 1024],
                            start=ysta,
                            stop=last,
                            skip_group_check=True,
                        )
                        mm = nc.tensor.matmul(
                            P[0:64, db + w0 : db + 512],
                            lhsT=ones_sb[:],
                            rhs=pt[:, w0:512],
                            start=first,
                            stop=last,
                            skip_group_check=True,
                        )
                        mm = nc.tensor.matmul(
                            P[64:128, db + w0 : db + 512],
                            lhsT=ones_sb[:],
                            rhs=pt[:, 512 + w0 : 1024],
                            start=ysta,
                            stop=last,
                            skip_group_check=True,
                        )
                        mm.then_inc(sAV, 1)

                def emit_outproj(r):
                    w(dWO, 128)
                    w(sA, (r + 1) * NBLK)  # block 15's mul: banks 6-7 free
                    order = [0, 4, 1, 5, 2, 6, 3, 7]
                    for qt in range(NST):
                        gq = r * NST + qt
                        base = 3072 + (qt % 2) * 512
                        qsl = slice(qt * 128, (qt + 1) * 128)
                        w(sOB, gq - 1)
                        for pos, dtk in enumerate(order):
                            w(dAT, 32 * (NHP * r + (dtk % 4) + 1))
                            mm = nc.tensor.matmul(
                                P[:, base : base + 512],
                                lhsT=ATB[dtk][:, qsl],
                                rhs=WOt[dtk][:],
                                start=(pos == 0),
                                stop=(pos == 7),
                                skip_group_check=True,
                            )
                        mm.then_inc(sOP, 1)

                # order: proj(0); attn(0); proj(1); outproj(0); attn(1); ...
                # — proj(r+1) between attention(r) and outproj(r) hides the
                # AllGather latency behind ~80us of projection matmuls.
                emit_proj(0)
                for r in range(reps):
                    emit_attention(r)
                    if r + 1 < reps:
                        emit_proj(r + 1)
                    emit_outproj(r)

            @blk.scalar
            def _(scalar):
                w = Waiter(scalar)
                for r in range(reps):
                    for i, (hp, qb, kt, nkt, delta) in enumerate(steps):
                        gi = r * NSTEPS + i
                        w0 = max(delta, 0)
                        s = i % 2
                        w(sPS, gi + 1)
                        w(sAV, gi - 5)
                        src = P[:, s * 1024 : (s + 1) * 1024]
                        dst = PT[i % 6][:, :]
                        if w0 == 0:
                            act = nc.scalar.activation(dst, src, AF.Exp, scale=SCALE)
                        else:
                            sv = src.rearrange("p (t c) -> p t c", t=2)[:, :, w0:512]
                            dv = dst.rearrange("p (t c) -> p t c", t=2)[:, :, w0:512]
                            act = nc.scalar.activation(dv, sv, AF.Exp, scale=SCALE)
                        act.then_inc(sEX, 1)

            @blk.vector
            def _(vector):
                w = Waiter(vector)
                nc.vector.memset(ones_sb[:], 1.0).then_inc(sON, 1)

                def emit_proj_copies(rr):
                    for j, item in enumerate(proj):
                        gj = rr * NPJ + j
                        bank = (j % 4) if item[0] != "v" else 4 + (j % 2)
                        pslc = slice(bank * 512, bank * 512 + 512)
                        w(sPJ, gj + 1)
                        if item[0] in ("q", "k"):
                            kind, hp, sb = item
                            if rr > 0:
                                w(sAV, (rr - 1) * NSTEPS + hp_last[hp] + 1)
                            dst = (QT if kind == "q" else KT)[hp]
                            ssl = slice(sb * 512, (sb + 1) * 512)
                            nc.vector.tensor_copy(dst[:, ssl], P[:, pslc]).then_inc(
                                sPJC, 1
                            )
                        else:
                            _, st = item
                            if rr > 0:
                                w(sAV, (rr - 1) * NSTEPS + vs_last[st] + 1)
                            nc.vector.tensor_copy(
                                VS[st][:, :], P[:, pslc]
                            ).then_inc(sPJC, 1)

                def emit_norms(r):
                    for i, (hp, qb, kt, nkt, delta) in enumerate(steps):
                        gi = r * NSTEPS + i
                        blk_idx = hp * NQB + qb
                        if kt == nkt - 1:
                            qsl = slice(qb * 512, (qb + 1) * 512)
                            ab = 2048 + (blk_idx % 2) * 1024
                            db = ab + 512
                            w(sAV, gi + 1)
                            if r > 0:
                                w(dCC, 16 * (NHP * (r - 1) + hp + 1))
                            nc.vector.reciprocal(RSB[:], P[:, db : db + 512])
                            nc.vector.tensor_mul(
                                ASBP[hp][:, qsl],
                                P[:, ab : ab + 512],
                                RSB[:],
                            ).then_inc(sA, 1)

                def emit_out_adds(r):
                    for qt in range(NST):
                        gq = r * NST + qt
                        w(sOP, gq + 1)
                        if gq >= 4:
                            w(dO[qt % 4], 16 * (r * (NST // 4) + qt // 4))
                        base = 3072 + (qt % 2) * 512
                        nc.vector.tensor_add(
                            OSB[qt % 4][:],
                            P[:, base : base + 512],
                            bob_sb[:],
                        ).then_inc(sOB, 1)

                emit_proj_copies(0)
                for r in range(reps):
                    emit_norms(r)
                    if r + 1 < reps:
                        emit_proj_copies(r + 1)
                    emit_out_adds(r)

    return nc


_cached = {}


def _get_program(S=S_FULL, reps=1):
    key = (S, reps)
    if key not in _cached:
        _cached[key] = build_program(S, reps)
    return _cached[key]


def make_in_maps(x, Wq, Wk, Wv, Wo, bo):
    bf = ml_dtypes.bfloat16
    ntri01 = (np.arange(896)[None, :] < (np.arange(128)[:, None] + 384)).astype(bf)
    negi01 = (np.eye(128) * -60000.0).astype(bf)
    x = np.asarray(x)
    # each batch's transposed activations feed both TP halves: build once
    xtb = [np.ascontiguousarray(x[b].T).astype(bf) for b in range(B)]
    in_maps = []
    for c in range(NCORES):
        b, p = divmod(c, 2)
        dsl = slice(p * DOWN, (p + 1) * DOWN)
        in_maps.append(
            {
                "xt": xtb[b],
                "wq": np.ascontiguousarray(np.asarray(Wq)[:, dsl]).astype(bf),
                "wk": np.ascontiguousarray(np.asarray(Wk)[:, dsl]).astype(bf),
                "wv": np.ascontiguousarray(np.asarray(Wv)[:, dsl]).astype(bf),
                "wo": np.ascontiguousarray(np.asarray(Wo)[:, dsl]).astype(bf),
                "bob": np.tile(np.asarray(bo, np.float32)[dsl], (128, 1)),
                "ntri": ntri01,
                "negi": negi01,
            }
        )
    return in_maps


def assemble(results, S):
    out = np.empty((B, S, D), np.float32)
    for c in range(NCORES):
        b, p = divmod(c, 2)
        out[b, :, p * DOWN : (p + 1) * DOWN] = results[c]["out"]
    return out


def kernel(**inputs):
    x = np.asarray(inputs["x"], np.float32)
    S = x.shape[1]
    nc = _get_program(S)
    in_maps = make_in_maps(
        x,
        inputs["Wq"],
        inputs["Wk"],
        inputs["Wv"],
        inputs["Wo"],
        inputs["bo"],
    )
    res = run_bass_kernel_spmd(nc, in_maps, core_ids=list(range(NCORES)))
    return assemble(res.results, S)
